# revision 37
# baseline (speedup 1.0000x reference)
"""Trainium2 Bass kernel for nn_MessagePassing (10-step 3x3 per-pixel-weighted stencil).

Algorithm (per core, one batch element):
  reference: nw = w / (sum_taps(w)+eps); 10x: x = sum_{di,dj} nw[di,dj] * shift(x, di, dj)

Device formulation (B-stationary, transpose-free): state lives as
[w=128 partitions, plane(h) x c] fp16.  For each output row r,
    out_r[wo, c] = sum_{di in 0..2} B[di, r]^T-free @ plane_{r+di-1}
realized as matmul(out, lhsT=B[di,r], rhs=plane) where
B[di,r][ws, wo] holds the RAW tap weight wt[3*di+dj, r, wo] at
ws = wo+dj-1 (host-side pure scatter/layout, fp16).  The three di
matmuls accumulate in PSUM; the output lands directly in state
orientation [wo, c] so no per-step transposes are needed.
Normalization (1/(sum9+eps), computed on device in fp32) is folded
into the PSUM->SBUF evacuation as a broadcast multiply: each psum
bank (8 rows) is multiplied by rec[w, r] (free-broadcast over c)
while casting f32 -> f16 into the next state buffer.

Layouts (per core):
  state: [w=128, (H+2) planes x 64 c] fp16 x2 ping-pong; plane 0 and
         plane 129 are zero pads (the 3x3 boundary).
  B:     chunk tiles [ws=128, (rows) x (di 3) x (wo 128)] fp16, sized
         per BCH_ROWS (small first for startup latency, 16-row later
         for DMA descriptor efficiency).
  rec:   [w=128, h=128] f32 = 1/(sum of 9 taps + eps), from wt9T.
  psum:  [128, 512] f32 = one bank = 8 output rows x 64 c.

Work is emitted in a greedy cross-step schedule (_schedule) so the PE
never idles waiting for DMA-gated step-1 rows while deeper steps have
runnable work.
"""

import numpy as np

C, H, W = 64, 128, 128
N_CORES = 8
STEPS = 10
EPS = 1e-5
RPB = 8            # output rows per psum bank / evac group
NG = H // RPB      # 16 groups per step
# B chunk tiling: small leading chunks (so the first matmuls can start
# as early as possible), growing to 16-row chunks late (bigger
# partition-lines amortize DMA descriptor overhead; arrival granularity
# stops mattering once the PE demand curve flattens).
BCH_ROWS = [2, 2, 4, 8, 8, 8, 8, 8, 16, 16, 16, 16, 16]
BCH_R0 = [sum(BCH_ROWS[:i]) for i in range(len(BCH_ROWS))]  # first row
NBC = len(BCH_ROWS)
assert sum(BCH_ROWS) == H
PL = H + 2         # state planes incl. zero pads
XCH = 4            # x groups per input DMA transfer
NXQ = NG // XCH    # 4 input transfers


def _schedule():
    """Greedy dependency-feasible (step, group) emission order.

    Engines execute their streams in program order, so the emission order IS
    the schedule.  Greedily emit whichever group can start earliest under a
    simple time model (PE ~0.82us/group, evacuation lands ~0.72us after a
    group's matmuls, B chunks arrive at ~294GB/s, x blocks early).  This
    deepens the step pyramid as fast as dependencies allow while step 1
    waits on DMA, instead of the fixed skew-2 wavefront.  Correctness never
    depends on the estimates - the Tile framework's semaphores enforce the
    real dependencies; estimates only shape the order.
    """
    # TT_US is the measured end-to-end lag from a group's last matmul to
    # its evacuation landing in SBUF (DVE queue + sem propagation).
    GROUP_US, TT_US, START, RATE = 0.82, 1.5, 8.6, 0.287
    arr, cum = [], 0.0
    for rows in BCH_ROWS:
        cum += rows * 3 * W * W * 2 / 1e6
        arr.append(START + cum / RATE)

    def chunk_of(row):
        for k in range(NBC):
            if BCH_R0[k] <= row < BCH_R0[k] + BCH_ROWS[k]:
                return k

    def b_ready(g):
        return max(arr[chunk_of(r)] for r in range(RPB * g, RPB * g + RPB))

    XARR = {0: 10.8, 1: 14.4, 2: 16.3, 3: 18.2}

    def x_ready(g):
        hi = min(RPB * g + RPB + 1, H - 1)
        return max(XARR[(RPB * g) // 32], XARR[hi // 32])

    done, order, progress = {}, [], [0] * STEPS
    cur = 11.0
    while len(order) < STEPS * NG:
        cands = []
        for s in range(STEPS):
            g = progress[s]
            if g >= NG:
                continue
            if s > 0 and progress[s - 1] < min(g + 2, NG):
                continue
            ready = b_ready(g)
            if s == 0:
                ready = max(ready, x_ready(g))
            else:
                for gg in (g - 1, g, g + 1):
                    if 0 <= gg < NG:
                        ready = max(ready, done[(s - 1, gg)] + TT_US)
            cands.append((max(ready, cur), s, g))
        tstart, s, g = min(cands)
        cur = tstart + GROUP_US
        done[(s, g)] = cur
        progress[s] += 1
        order.append((s, g))
    return order


def build_nc():
    import concourse.mybir as mybir
    from concourse import bacc
    from concourse.tile import TileContext

    f32 = mybir.dt.float32
    f16 = mybir.dt.float16

    nc = bacc.Bacc(trn_type="TRN2", target_bir_lowering=False, debug=False)
    # chunk-major layouts: each DMA chunk is a contiguous block of DRAM
    xT = nc.dram_tensor("xT", [NXQ * W, XCH * RPB * C], f16,
                        kind="ExternalInput").ap()
    braws = [nc.dram_tensor(f"braw{k}", [W, BCH_ROWS[k] * 3 * W], f16,
                            kind="ExternalInput").ap() for k in range(NBC)]
    wt9T = nc.dram_tensor("wt9T", [W, H * 9], f32, kind="ExternalInput").ap()
    yT = nc.dram_tensor("yT", [NG * W, RPB * C], f16, kind="ExternalOutput").ap()

    with TileContext(nc) as tc:
        with (
            tc.tile_pool(name="per", bufs=1) as per,
            tc.tile_pool(name="ps", bufs=8, space="PSUM") as ps,
        ):
            # ---- persistent SBUF ----
            Bt = [per.tile([W, BCH_ROWS[k] * 3 * W], f16, tag=f"B{k}",
                           name=f"B{k}") for k in range(NBC)]
            st = [per.tile([W, PL * C], f16, tag=f"st{s}", name=f"st{s}")
                  for s in range(2)]
            wt_sb = per.tile([W, H * 9], f32, tag="wt")
            sum9 = per.tile([W, H], f32, tag="sum9")
            rec = per.tile([W, H], f32, tag="rec")

            # ---- loads ----
            # B chunks stream on the scalar HWDGE queue (one queue: two
            # concurrent HWDGE queues split packets and lose aggregate
            # rate); x + wt9T on the sync queue in a few large transfers.
            for k in range(NBC):
                nc.scalar.dma_start(out=Bt[k][:], in_=braws[k])

            # x: first group alone (it gates the first evacuation), then
            # the rest of its 4-group block, then two 6-group blocks
            nc.sync.dma_start(
                out=st[0][:, C:(RPB + 1) * C],
                in_=xT[0:W, 0:RPB * C],
            )
            nc.sync.dma_start(
                out=st[0][:, (RPB + 1) * C:(XCH * RPB + 1) * C],
                in_=xT[0:W, RPB * C:],
            )
            nc.sync.dma_start(out=wt_sb[:], in_=wt9T)
            for q in range(1, NXQ):
                nc.sync.dma_start(
                    out=st[0][:, (q * XCH * RPB + 1) * C:
                              ((q + 1) * XCH * RPB + 1) * C],
                    in_=xT[q * W:(q + 1) * W, :],
                )
            # zero pads (both ping-pong buffers, never written again)
            for s in range(2):
                nc.vector.memset(st[s][:, 0:C], 0.0)
                nc.vector.memset(st[s][:, (H + 1) * C:PL * C], 0.0)

            # ---- rec = 1/(sum9 + eps), in [w, h] orientation ----
            nc.vector.tensor_reduce(
                out=sum9[:].unsqueeze(2),
                in_=wt_sb[:].rearrange("p (h t) -> p h t", t=9),
                axis=mybir.AxisListType.X,
                op=mybir.AluOpType.add,
            )
            nc.vector.tensor_scalar_add(out=sum9[:], in0=sum9[:],
                                        scalar1=float(EPS))
            nc.vector.reciprocal(out=rec[:], in_=sum9[:])

            # ---- helper ----
            row2chunk = {}
            for k in range(NBC):
                for rr in range(BCH_ROWS[k]):
                    row2chunk[BCH_R0[k] + rr] = (k, rr)

            def bmat(r, di):  # stationary [ws=128, wo=128] for (row r, di)
                k, rr = row2chunk[r]
                off = (rr * 3 + di) * W
                return Bt[k][:, off:off + W]

            # ---- the 10 steps, emitted in greedy dependency order ----
            # (see _schedule: step s group g needs step s-1 groups <= g+1
            # evacuated, plus its B chunks and, for step 0, its x rows)
            def do_group(s, g):
                src = st[s % 2]
                dst = st[(s + 1) % 2]
                pst = ps.tile([W, RPB * C], f32, tag="ps", name="ps")
                for rr in range(RPB):
                    r = g * RPB + rr
                    for di in range(3):
                        # source plane index r+di (zero pads at the ends
                        # make the boundary rows correct)
                        nc.tensor.matmul(
                            out=pst[:, rr * C:(rr + 1) * C],
                            lhsT=bmat(r, di),
                            rhs=src[:, (r + di) * C:(r + di + 1) * C],
                            start=(di == 0),
                            stop=(di == 2),
                            # 8 independent row-groups share this bank;
                            # per-element pending-zero semantics make
                            # this correct but the sim's group checker
                            # conflates col ranges within a bank.
                            skip_group_check=True,
                        )
                # fused evacuation: cast f32->f16 AND normalize by
                # rec[w, r] (broadcast over c)
                in0 = pst[:].rearrange("p (r c) -> p r c", r=RPB)
                in1 = (rec[:, g * RPB:(g + 1) * RPB]
                       .unsqueeze(2).broadcast_to([W, RPB, C]))
                outap = dst[:, (g * RPB + 1) * C:(g * RPB + RPB + 1) * C
                            ].rearrange("p (r c) -> p r c", r=RPB)
                nc.vector.tensor_mul(out=outap, in0=in0, in1=in1)
                if s == STEPS - 1:
                    # stream the finished group straight out (fp16, HWDGE)
                    nc.sync.dma_start(
                        out=yT[g * W:(g + 1) * W, :],
                        in_=dst[:, (g * RPB + 1) * C:(g * RPB + RPB + 1) * C],
                    )

            for s, g in _schedule():
                do_group(s, g)

    if not nc.is_finalized():
        nc.finalize()
    return nc


def host_prep(inp_i, wt_i):
    """Per-core host-side layout transforms (+ the fp16 quantization the
    device pipeline uses; the f16->f32 widening on output is exact)."""
    # xT chunk-major: [q, w, (32 rows) x c] = x[c, 32q+rr, w]
    xT = (inp_i.transpose(2, 1, 0)          # [w, h, c]
          .reshape(W, NXQ, XCH * RPB * C)
          .transpose(1, 0, 2)
          .reshape(NXQ * W, XCH * RPB * C))
    # braw[ws, h, di, wo] = wt_i[3*di+dj, h, wo] with ws = wo+dj-1
    braw = np.zeros((W, H, 3, W), dtype=np.float16)
    wo = np.arange(W)
    for di in range(3):
        for dj in range(3):
            ws = wo + dj - 1
            m = (ws >= 0) & (ws < W)
            braw[ws[m], :, di, wo[m]] = wt_i[3 * di + dj][:, wo[m]].T.astype(
                np.float16
            )
    # per-chunk contiguous blocks
    out = {}
    for k in range(NBC):
        r0, nr = BCH_R0[k], BCH_ROWS[k]
        out[f"braw{k}"] = np.ascontiguousarray(
            braw[:, r0:r0 + nr]).reshape(W, nr * 3 * W)
    # wt9T[w, h, t] = wt_i[t, h, w]
    wt9T = np.ascontiguousarray(wt_i.transpose(2, 1, 0)).reshape(W, H * 9)
    out["xT"] = np.ascontiguousarray(xT).astype(np.float16)
    out["wt9T"] = wt9T.astype(np.float32)
    return out


def unpack(yT):
    # yT[g, w, r, c] -> [c, h=8g+r, w]
    return (yT.reshape(NG, W, RPB, C)
            .transpose(3, 0, 2, 1)
            .reshape(C, H, W)
            .astype(np.float32))


LAST_RESULTS = None  # BassKernelResults of the most recent kernel() call


def kernel(**inputs):
    import os
    from concourse.bass_utils import run_bass_kernel_spmd

    global LAST_RESULTS
    inp = np.asarray(inputs["input"], dtype=np.float32)
    wt = np.asarray(inputs["weight"], dtype=np.float32)
    n = inp.shape[0]
    in_maps = [host_prep(inp[i], wt[i]) for i in range(n)]
    nc = build_nc()
    trace = bool(int(os.environ.get("MP_TRACE", "0")))
    res = run_bass_kernel_spmd(
        nc, in_maps, core_ids=list(range(n)), trace=trace
    )
    LAST_RESULTS = res
    out = np.stack([unpack(r["yT"]) for r in res.results])
    return out.astype(np.float32)


if __name__ == "__main__":
    nc = build_nc()
    print("built ok")


# revision 41
# speedup vs baseline: 1.0030x; 1.0030x over previous
"""Trainium2 Bass kernel for nn_MessagePassing (10-step 3x3 per-pixel-weighted stencil).

Algorithm (per core, one batch element):
  reference: nw = w / (sum_taps(w)+eps); 10x: x = sum_{di,dj} nw[di,dj] * shift(x, di, dj)

Device formulation (B-stationary, transpose-free): state lives as
[w=128 partitions, plane(h) x c] fp16.  For each output row r,
    out_r[wo, c] = sum_{di in 0..2} B[di, r]^T-free @ plane_{r+di-1}
realized as matmul(out, lhsT=B[di,r], rhs=plane) where
B[di,r][ws, wo] holds the RAW tap weight wt[3*di+dj, r, wo] at
ws = wo+dj-1 (host-side pure scatter/layout, fp16).  The three di
matmuls accumulate in PSUM; the output lands directly in state
orientation [wo, c] so no per-step transposes are needed.
Normalization (1/(sum9+eps), computed on device in fp32) is folded
into the PSUM->SBUF evacuation as a broadcast multiply: each psum
bank (8 rows) is multiplied by rec[w, r] (free-broadcast over c)
while casting f32 -> f16 into the next state buffer.

Layouts (per core):
  state: [w=128, (H+2) planes x 64 c] fp16 x2 ping-pong; plane 0 and
         plane 129 are zero pads (the 3x3 boundary).
  B:     chunk tiles [ws=128, (rows) x (di 3) x (wo 128)] fp16, sized
         per BCH_ROWS (small first for startup latency, 16-row later
         for DMA descriptor efficiency).
  rec:   [w=128, h=128] f32 = 1/(sum of 9 taps + eps), from wt9T.
  psum:  [128, 512] f32 = one bank = 8 output rows x 64 c.

Work is emitted in a greedy cross-step schedule (_schedule) so the PE
never idles waiting for DMA-gated step-1 rows while deeper steps have
runnable work.
"""

import numpy as np

C, H, W = 64, 128, 128
N_CORES = 8
STEPS = 10
EPS = 1e-5
RPB = 8            # output rows per psum bank / evac group
NG = H // RPB      # 16 groups per step
# B chunk tiling: small leading chunks (so the first matmuls can start
# as early as possible), growing to 16-row chunks late (bigger
# partition-lines amortize DMA descriptor overhead; arrival granularity
# stops mattering once the PE demand curve flattens).
BCH_ROWS = [2, 2, 4, 8, 8, 8, 8, 8, 16, 16, 16, 16, 16]
BCH_R0 = [sum(BCH_ROWS[:i]) for i in range(len(BCH_ROWS))]  # first row
NBC = len(BCH_ROWS)
assert sum(BCH_ROWS) == H
PL = H + 2         # state planes incl. zero pads
XCH = 4            # x groups per input DMA transfer
NXQ = NG // XCH    # 4 input transfers


def _schedule():
    """Greedy dependency-feasible (step, group) emission order.

    Engines execute their streams in program order, so the emission order IS
    the schedule.  Greedily emit whichever group can start earliest under a
    simple time model (PE ~0.82us/group, evacuation lands ~0.72us after a
    group's matmuls, B chunks arrive at ~294GB/s, x blocks early).  This
    deepens the step pyramid as fast as dependencies allow while step 1
    waits on DMA, instead of the fixed skew-2 wavefront.  Correctness never
    depends on the estimates - the Tile framework's semaphores enforce the
    real dependencies; estimates only shape the order.
    """
    # TT_US is the measured end-to-end lag from a group's last matmul to
    # its evacuation landing in SBUF (DVE queue + sem propagation).
    GROUP_US, TT_US, START, RATE = 0.82, 1.5, 8.6, 0.287
    arr, cum = [], 0.0
    for rows in BCH_ROWS:
        cum += rows * 3 * W * W * 2 / 1e6
        arr.append(START + cum / RATE)

    def chunk_of(row):
        for k in range(NBC):
            if BCH_R0[k] <= row < BCH_R0[k] + BCH_ROWS[k]:
                return k

    def b_ready(g):
        return max(arr[chunk_of(r)] for r in range(RPB * g, RPB * g + RPB))

    XARR = {0: 10.8, 1: 14.4, 2: 16.3, 3: 18.2}

    def x_ready(g):
        hi = min(RPB * g + RPB + 1, H - 1)
        return max(XARR[(RPB * g) // 32], XARR[hi // 32])

    done, order, progress = {}, [], [0] * STEPS
    cur = 11.0
    while len(order) < STEPS * NG:
        cands = []
        for s in range(STEPS):
            g = progress[s]
            if g >= NG:
                continue
            if s > 0 and progress[s - 1] < min(g + 2, NG):
                continue
            ready = b_ready(g)
            if s == 0:
                ready = max(ready, x_ready(g))
            else:
                for gg in (g - 1, g, g + 1):
                    if 0 <= gg < NG:
                        ready = max(ready, done[(s - 1, gg)] + TT_US)
            cands.append((max(ready, cur), s, g))
        tstart, s, g = min(cands)
        cur = tstart + GROUP_US
        done[(s, g)] = cur
        progress[s] += 1
        order.append((s, g))
    return order


def build_nc():
    import concourse.mybir as mybir
    from concourse import bacc
    from concourse.tile import TileContext

    f32 = mybir.dt.float32
    f16 = mybir.dt.float16

    nc = bacc.Bacc(trn_type="TRN2", target_bir_lowering=False, debug=False)
    # chunk-major layouts: each DMA chunk is a contiguous block of DRAM
    xT = nc.dram_tensor("xT", [NXQ * W, XCH * RPB * C], f16,
                        kind="ExternalInput").ap()
    braws = [nc.dram_tensor(f"braw{k}", [W, BCH_ROWS[k] * 3 * W], f16,
                            kind="ExternalInput").ap() for k in range(NBC)]
    wt9T = nc.dram_tensor("wt9T", [W, H * 9], f32, kind="ExternalInput").ap()
    yT = nc.dram_tensor("yT", [(NG // 2) * W, 2 * RPB * C], f16,
                        kind="ExternalOutput").ap()

    with TileContext(nc) as tc:
        with (
            tc.tile_pool(name="per", bufs=1) as per,
            tc.tile_pool(name="ps", bufs=8, space="PSUM") as ps,
        ):
            # ---- persistent SBUF ----
            Bt = [per.tile([W, BCH_ROWS[k] * 3 * W], f16, tag=f"B{k}",
                           name=f"B{k}") for k in range(NBC)]
            st = [per.tile([W, PL * C], f16, tag=f"st{s}", name=f"st{s}")
                  for s in range(2)]
            wt_sb = per.tile([W, H * 9], f32, tag="wt")
            sum9 = per.tile([W, H], f32, tag="sum9")
            rec = per.tile([W, H], f32, tag="rec")

            # ---- loads ----
            # B chunks stream on the scalar HWDGE queue (one queue: two
            # concurrent HWDGE queues split packets and lose aggregate
            # rate); x + wt9T on the sync queue in a few large transfers.
            for k in range(NBC):
                nc.scalar.dma_start(out=Bt[k][:], in_=braws[k])

            # x: first group alone (it gates the first evacuation), then
            # the rest of its 4-group block, then two 6-group blocks
            nc.sync.dma_start(
                out=st[0][:, C:(RPB + 1) * C],
                in_=xT[0:W, 0:RPB * C],
            )
            nc.sync.dma_start(
                out=st[0][:, (RPB + 1) * C:(XCH * RPB + 1) * C],
                in_=xT[0:W, RPB * C:],
            )
            nc.sync.dma_start(out=wt_sb[:], in_=wt9T)
            for q in range(1, NXQ):
                nc.sync.dma_start(
                    out=st[0][:, (q * XCH * RPB + 1) * C:
                              ((q + 1) * XCH * RPB + 1) * C],
                    in_=xT[q * W:(q + 1) * W, :],
                )
            # zero pads (both ping-pong buffers, never written again)
            for s in range(2):
                nc.vector.memset(st[s][:, 0:C], 0.0)
                nc.vector.memset(st[s][:, (H + 1) * C:PL * C], 0.0)

            # ---- rec = 1/(sum9 + eps), in [w, h] orientation ----
            nc.vector.tensor_reduce(
                out=sum9[:].unsqueeze(2),
                in_=wt_sb[:].rearrange("p (h t) -> p h t", t=9),
                axis=mybir.AxisListType.X,
                op=mybir.AluOpType.add,
            )
            nc.vector.tensor_scalar_add(out=sum9[:], in0=sum9[:],
                                        scalar1=float(EPS))
            nc.vector.reciprocal(out=rec[:], in_=sum9[:])

            # ---- helper ----
            row2chunk = {}
            for k in range(NBC):
                for rr in range(BCH_ROWS[k]):
                    row2chunk[BCH_R0[k] + rr] = (k, rr)

            def bmat(r, di):  # stationary [ws=128, wo=128] for (row r, di)
                k, rr = row2chunk[r]
                off = (rr * 3 + di) * W
                return Bt[k][:, off:off + W]

            # ---- the 10 steps, emitted in greedy dependency order ----
            # (see _schedule: step s group g needs step s-1 groups <= g+1
            # evacuated, plus its B chunks and, for step 0, its x rows)
            def do_group(s, g):
                src = st[s % 2]
                dst = st[(s + 1) % 2]
                pst = ps.tile([W, RPB * C], f32, tag="ps", name="ps")
                for rr in range(RPB):
                    r = g * RPB + rr
                    for di in range(3):
                        # source plane index r+di (zero pads at the ends
                        # make the boundary rows correct)
                        nc.tensor.matmul(
                            out=pst[:, rr * C:(rr + 1) * C],
                            lhsT=bmat(r, di),
                            rhs=src[:, (r + di) * C:(r + di + 1) * C],
                            start=(di == 0),
                            stop=(di == 2),
                            # 8 independent row-groups share this bank;
                            # per-element pending-zero semantics make
                            # this correct but the sim's group checker
                            # conflates col ranges within a bank.
                            skip_group_check=True,
                        )
                # fused evacuation: cast f32->f16 AND normalize by
                # rec[w, r] (broadcast over c)
                in0 = pst[:].rearrange("p (r c) -> p r c", r=RPB)
                in1 = (rec[:, g * RPB:(g + 1) * RPB]
                       .unsqueeze(2).broadcast_to([W, RPB, C]))
                outap = dst[:, (g * RPB + 1) * C:(g * RPB + RPB + 1) * C
                            ].rearrange("p (r c) -> p r c", r=RPB)
                nc.vector.tensor_mul(out=outap, in0=in0, in1=in1)
                if s == STEPS - 1 and g % 2 == 1:
                    # stream finished group pairs straight out (fp16, HWDGE,
                    # pair-major contiguous blocks)
                    pr = g // 2
                    nc.sync.dma_start(
                        out=yT[pr * W:(pr + 1) * W, :],
                        in_=dst[:, ((g - 1) * RPB + 1) * C:
                                (g * RPB + RPB + 1) * C],
                    )

            for s, g in _schedule():
                do_group(s, g)

    if not nc.is_finalized():
        nc.finalize()
    return nc


def host_prep(inp_i, wt_i):
    """Per-core host-side layout transforms (+ the fp16 quantization the
    device pipeline uses; the f16->f32 widening on output is exact)."""
    # xT chunk-major: [q, w, (32 rows) x c] = x[c, 32q+rr, w]
    xT = (inp_i.transpose(2, 1, 0)          # [w, h, c]
          .reshape(W, NXQ, XCH * RPB * C)
          .transpose(1, 0, 2)
          .reshape(NXQ * W, XCH * RPB * C))
    # braw[ws, h, di, wo] = wt_i[3*di+dj, h, wo] with ws = wo+dj-1
    braw = np.zeros((W, H, 3, W), dtype=np.float16)
    wo = np.arange(W)
    for di in range(3):
        for dj in range(3):
            ws = wo + dj - 1
            m = (ws >= 0) & (ws < W)
            braw[ws[m], :, di, wo[m]] = wt_i[3 * di + dj][:, wo[m]].T.astype(
                np.float16
            )
    # per-chunk contiguous blocks
    out = {}
    for k in range(NBC):
        r0, nr = BCH_R0[k], BCH_ROWS[k]
        out[f"braw{k}"] = np.ascontiguousarray(
            braw[:, r0:r0 + nr]).reshape(W, nr * 3 * W)
    # wt9T[w, h, t] = wt_i[t, h, w]
    wt9T = np.ascontiguousarray(wt_i.transpose(2, 1, 0)).reshape(W, H * 9)
    out["xT"] = np.ascontiguousarray(xT).astype(np.float16)
    out["wt9T"] = wt9T.astype(np.float32)
    return out


def unpack(yT):
    # yT[pair, w, rr(16), c] -> [c, h=16*pair+rr, w]
    return (yT.reshape(NG // 2, W, 2 * RPB, C)
            .transpose(3, 0, 2, 1)
            .reshape(C, H, W)
            .astype(np.float32))


LAST_RESULTS = None  # BassKernelResults of the most recent kernel() call


def kernel(**inputs):
    import os
    from concourse.bass_utils import run_bass_kernel_spmd

    global LAST_RESULTS
    inp = np.asarray(inputs["input"], dtype=np.float32)
    wt = np.asarray(inputs["weight"], dtype=np.float32)
    n = inp.shape[0]
    in_maps = [host_prep(inp[i], wt[i]) for i in range(n)]
    nc = build_nc()
    trace = bool(int(os.environ.get("MP_TRACE", "0")))
    res = run_bass_kernel_spmd(
        nc, in_maps, core_ids=list(range(n)), trace=trace
    )
    LAST_RESULTS = res
    out = np.stack([unpack(r["yT"]) for r in res.results])
    return out.astype(np.float32)


if __name__ == "__main__":
    nc = build_nc()
    print("built ok")
